# revision 22
# baseline (speedup 1.0000x reference)
"""MoIE transformer block — Bass/Tile kernel for 8 Trainium2 NeuronCores.

Contract: kernel(**inputs) takes FULL (unsharded) inputs (numpy, fp32) and
returns the FULL [4, 2048, 1024] fp32 output.

Sharding (data-parallel, 2 cores per batch, causally balanced):
  core c -> batch b = c//2, half h = c%2. The core owns query tiles
  g = 2j+h (j = 0..7) of 128 rows each. Host passes x row-PERMUTED:
  [my 8 tiles | partner 8 tiles], so the device program is identical for
  all cores (pure SPMD); all per-core differences live in input data
  (x permutation + attention masks).

Device pipeline per core (all matmuls bf16 with fp32 PSUM accumulate):
  ln (bn_stats/bn_aggr, fp32) -> ln1 bf16 (normal + PE-transposed layouts)
  v,k,q routed-expert branches:
      match' = ln1 @ protoT/32 (- cost),  comp = silu(ln1 @ muT + b)
      out = (match'-cost) > 0 ? comp*relu(match'-cost) : ln1   [copy_predicated]
      (q branch additionally folds the attention 1/sqrt(D)=1/32 scale)
  causal attention over 256-wide key blocks (mine + partner regions, last
  blocks masked via additive input masks), softmax via ACT Exp + accum_out,
  p transposed on PE for the pv matmul.
  o branch on attn, + residual x, DMA out.
"""

import sys

sys.path.insert(0, "/opt/trn_rl_repo")

from contextlib import ExitStack

import numpy as np
import ml_dtypes

import concourse.bass as bass
import concourse.bacc as bacc
import concourse.tile as tile
from concourse import mybir, masks
from concourse.bass_utils import run_bass_kernel_spmd

BF16 = ml_dtypes.bfloat16
F32 = mybir.dt.float32
BF = mybir.dt.bfloat16
AF = mybir.ActivationFunctionType
ALU = mybir.AluOpType
AX = mybir.AxisListType

P = 128
D = 1024
ND = D // P          # 8 feature chunks
NT = 16              # row tiles per batch (2048/128)
NJ = 8               # query tiles per core
NEG = -1e9
EPS_LN = 1e-5
SCALE = 1.0 / 32.0   # 1/sqrt(D)


def _nblocks(j):
    """256-wide key blocks per region (mine / partner) for query tile j."""
    return (j + 2) // 2  # ceil((j+1)/2)


def _build(gb_trivial, bq, bk, bv, bo):
    nc = bacc.Bacc("TRN2", target_bir_lowering=False, debug=False, num_devices=8)

    x_d = nc.dram_tensor("x", [2048, D], F32, kind="ExternalInput")
    wp_d, wm_d, bias_d = {}, {}, {}
    for br in "qkvo":
        wp_d[br] = nc.dram_tensor(f"wp_{br}", [D, D], BF, kind="ExternalInput")
        wm_d[br] = nc.dram_tensor(f"wm_{br}", [D, D], BF, kind="ExternalInput")
    ncq_d = nc.dram_tensor("ncost_q", [D], F32, kind="ExternalInput")
    nck_d = nc.dram_tensor("ncost_k", [D], F32, kind="ExternalInput")
    cv_d = nc.dram_tensor("cost_v", [D], F32, kind="ExternalInput")
    co_d = nc.dram_tensor("cost_o", [D], F32, kind="ExternalInput")
    mme_d = nc.dram_tensor("mask_m_even", [P, 256], F32, kind="ExternalInput")
    mmo_d = nc.dram_tensor("mask_m_odd", [P, 256], F32, kind="ExternalInput")
    mpe_d = nc.dram_tensor("mask_p_even", [P, 256], F32, kind="ExternalInput")
    mpo_d = nc.dram_tensor("mask_p_odd", [P, 256], F32, kind="ExternalInput")
    if not gb_trivial:
        g_d = nc.dram_tensor("ln_g", [D], F32, kind="ExternalInput")
        b_d = nc.dram_tensor("ln_b", [D], F32, kind="ExternalInput")
    for br, flag in zip("qkvo", (bq, bk, bv, bo)):
        if flag:
            bias_d[br] = nc.dram_tensor(f"bias_{br}", [D], F32, kind="ExternalInput")
    out_d = nc.dram_tensor("out", [NJ * P, D], F32, kind="ExternalOutput")

    def bcast_row(dram_h):
        return bass.AP(tensor=dram_h[:].tensor, offset=0, ap=[[0, P], [1, D]])

    with tile.TileContext(nc) as tc, ExitStack() as top:
        const = top.enter_context(tc.tile_pool(name="const", bufs=1))
        idf = const.tile([P, P], F32, tag="idf")
        idb = const.tile([P, P], BF, tag="idb")
        masks.make_identity(nc, idf[:])
        masks.make_identity(nc, idb[:])

        ncq = const.tile([P, ND], F32, tag="ncq")
        nc.sync.dma_start(out=ncq, in_=ncq_d[:].rearrange("(dc p) -> p dc", p=P))
        nck = const.tile([P, ND], F32, tag="nck")
        nc.sync.dma_start(out=nck, in_=nck_d[:].rearrange("(dc p) -> p dc", p=P))
        cvb = const.tile([P, D], F32, tag="cvb")
        nc.gpsimd.dma_start(out=cvb, in_=bcast_row(cv_d))
        cob = const.tile([P, D], F32, tag="cob")
        nc.gpsimd.dma_start(out=cob, in_=bcast_row(co_d))

        eps_t = const.tile([P, 1], F32, tag="eps")
        nc.vector.memset(eps_t, EPS_LN)

        mme = const.tile([P, 256], F32, tag="mme")
        nc.sync.dma_start(out=mme, in_=mme_d[:])
        mmo = const.tile([P, 256], F32, tag="mmo")
        nc.sync.dma_start(out=mmo, in_=mmo_d[:])
        mpe = const.tile([P, 256], F32, tag="mpe")
        nc.sync.dma_start(out=mpe, in_=mpe_d[:])
        mpo = const.tile([P, 256], F32, tag="mpo")
        nc.sync.dma_start(out=mpo, in_=mpo_d[:])

        if not gb_trivial:
            gbg = const.tile([P, D], F32, tag="gbg")
            nc.gpsimd.dma_start(out=gbg, in_=bcast_row(g_d))
            gbb = const.tile([P, D], F32, tag="gbb")
            nc.gpsimd.dma_start(out=gbb, in_=bcast_row(b_d))
        bias_t = {}
        for br in "qk":
            if br in bias_d:
                t = const.tile([P, ND], F32, tag=f"bias_{br}")
                nc.sync.dma_start(
                    out=t, in_=bias_d[br][:].rearrange("(dc p) -> p dc", p=P)
                )
                bias_t[br] = t
        for br in "vo":
            if br in bias_d:
                t = const.tile([P, D], F32, tag=f"bias_{br}")
                nc.gpsimd.dma_start(out=t, in_=bcast_row(bias_d[br]))
                bias_t[br] = t

        # ---- persistent activation tensors ----
        # SBUF pools release in stack order per side; lnT/lnbf/attn live on
        # the right stack (their lifetimes straddle the left-stack phases).
        es_lnT = ExitStack()
        lnT = es_lnT.enter_context(
            tc.tile_pool(name="lnT", bufs=1, side="right")
        ).tile([P, ND, 2048], BF, tag="lnT")
        es_lnbf = ExitStack()
        lnbf = es_lnbf.enter_context(
            tc.tile_pool(name="lnbf", bufs=1, side="right")
        ).tile([P, NT, D], BF, tag="lnbf")
        es_v = ExitStack()
        es_kT = ExitStack()
        es_qT = ExitStack()
        es_attn = ExitStack()

        # ================= phase A: layernorm + transpose =================
        with ExitStack() as esA:
            scrA = esA.enter_context(tc.tile_pool(name="scrA", bufs=3))
            psA = esA.enter_context(tc.tile_pool(name="psA", bufs=2, space="PSUM"))
            for rt in range(NT):
                xt = scrA.tile([P, D], F32, tag="xt")
                nc.sync.dma_start(out=xt, in_=x_d[rt * P : (rt + 1) * P, :])
                stats = scrA.tile([P, 2, 6], F32, tag="st")
                xr = xt[:].rearrange("p (n f) -> p n f", f=512)
                for sg in range(2):
                    nc.vector.bn_stats(out=stats[:, sg, :], in_=xr[:, sg, :])
                mv = scrA.tile([P, 2], F32, tag="mv")
                nc.vector.bn_aggr(out=mv, in_=stats)
                std = scrA.tile([P, 1], F32, tag="sd")
                nc.scalar.activation(
                    out=std, in_=mv[:, 1:2], func=AF.Sqrt, bias=eps_t, scale=1.0
                )
                rstd = scrA.tile([P, 1], F32, tag="rs")
                nc.vector.reciprocal(out=rstd, in_=std)
                lnf = scrA.tile([P, D], F32, tag="lnf")
                nc.vector.tensor_scalar(
                    out=lnf,
                    in0=xt,
                    scalar1=mv[:, 0:1],
                    scalar2=rstd,
                    op0=ALU.subtract,
                    op1=ALU.mult,
                )
                if not gb_trivial:
                    nc.vector.tensor_tensor(out=lnf, in0=lnf, in1=gbg, op=ALU.mult)
                    nc.vector.tensor_tensor(out=lnf, in0=lnf, in1=gbb, op=ALU.add)
                nc.scalar.copy(out=lnbf[:, rt, :], in_=lnf)
                for half in range(2):
                    trp = psA.tile([P, 512], F32, tag="tr")
                    for t in range(4):
                        dc = half * 4 + t
                        nc.tensor.transpose(
                            out=trp[:, t * P : (t + 1) * P],
                            in_=lnf[:, dc * P : (dc + 1) * P],
                            identity=idf,
                        )
                    nc.vector.tensor_copy(
                        out=lnT[:, half * 4 : (half + 1) * 4, rt * P : (rt + 1) * P],
                        in_=trp[:].rearrange("p (a b) -> p a b", b=P),
                    )

        # weights pool: 3 slots of [P, ND, D] bf16 (16KB/partition each)
        p_w = top.enter_context(tc.tile_pool(name="wpool", bufs=3))

        def load_w(dram_h):
            t = p_w.tile([P, ND, D], BF, tag="w")
            nc.sync.dma_start(
                out=t, in_=dram_h[:].rearrange("(dc p) f -> p dc f", p=P)
            )
            return t

        # ============ phases B,C,D: v, k, q branches ============
        es_scrB = ExitStack()
        es_psB = ExitStack()
        if True:
            scr = es_scrB.enter_context(tc.tile_pool(name="scrB", bufs=3))
            psB = es_psB.enter_context(tc.tile_pool(name="psB", bufs=4, space="PSUM"))

            def mm_acc(ps, lhsT_fn, rhs_fn):
                for dc in range(ND):
                    nc.tensor.matmul(
                        ps,
                        lhsT_fn(dc),
                        rhs_fn(dc),
                        start=(dc == 0),
                        stop=(dc == ND - 1),
                    )

            # ---- v branch (normal orientation, 16 row tiles) ----
            vN = es_v.enter_context(tc.tile_pool(name="vN", bufs=1)).tile(
                [P, NT, D], BF, tag="vN"
            )
            wpv, wmv = load_w(wp_d["v"]), load_w(wm_d["v"])
            for rt in range(NT):
                for nh in range(2):
                    sl = slice(nh * 512, (nh + 1) * 512)
                    rl = psB.tile([P, 512], F32, tag="mm")
                    cp = psB.tile([P, 512], F32, tag="mm")
                    mm_acc(rl, lambda dc: lnT[:, dc, rt * P : (rt + 1) * P],
                           lambda dc: wpv[:, dc, sl])
                    mm_acc(cp, lambda dc: lnT[:, dc, rt * P : (rt + 1) * P],
                           lambda dc: wmv[:, dc, sl])
                    if "v" in bias_t:
                        nc.vector.tensor_tensor(
                            out=cp, in0=cp, in1=bias_t["v"][:, sl], op=ALU.add
                        )
                    nc.vector.tensor_tensor(
                        out=rl, in0=rl, in1=cvb[:, sl], op=ALU.subtract
                    )
                    mrl = scr.tile([P, 512], F32, tag="mrl")
                    nc.scalar.activation(out=mrl, in_=rl, func=AF.Relu, bias=0.0)
                    comp = scr.tile([P, 512], F32, tag="comp")
                    nc.scalar.activation(out=comp, in_=cp, func=AF.Silu, bias=0.0)
                    dst = vN[:, rt, sl]
                    nc.scalar.copy(out=dst, in_=lnbf[:, rt, sl])
                    t = scr.tile([P, 512], BF, tag="t")
                    nc.vector.tensor_mul(out=t, in0=comp, in1=mrl)
                    msk = scr.tile([P, 512], mybir.dt.uint8, tag="msk")
                    nc.vector.tensor_scalar(
                        out=msk, in0=mrl, scalar1=0.0, scalar2=None, op0=ALU.is_gt
                    )
                    nc.vector.copy_predicated(out=dst, mask=msk, data=t)
            es_lnbf.close()

            # ---- k, q branches (transposed orientation) ----
            def t_branch(out_t, wp, wm, ncost, bias, ncols, mscale, pscale):
                for ft in range(ND):
                    for cc in range(ncols // 512):
                        sl = slice(cc * 512, (cc + 1) * 512)
                        rl = psB.tile([P, 512], F32, tag="mm")
                        cp = psB.tile([P, 512], F32, tag="mm")
                        mm_acc(rl, lambda dc: wp[:, dc, ft * P : (ft + 1) * P],
                               lambda dc: lnT[:, dc, sl])
                        mm_acc(cp, lambda dc: wm[:, dc, ft * P : (ft + 1) * P],
                               lambda dc: lnT[:, dc, sl])
                        mrl = scr.tile([P, 512], F32, tag="mrl")
                        nc.scalar.activation(
                            out=mrl, in_=rl, func=AF.Relu,
                            bias=ncost[:, ft : ft + 1], scale=mscale,
                        )
                        comp = scr.tile([P, 512], F32, tag="comp")
                        nc.scalar.activation(
                            out=comp, in_=cp, func=AF.Silu,
                            bias=(bias[:, ft : ft + 1] if bias is not None else 0.0),
                        )
                        dst = out_t[:, ft, sl]
                        if pscale == 1.0:
                            nc.scalar.copy(out=dst, in_=lnT[:, ft, sl])
                        else:
                            nc.scalar.activation(
                                out=dst, in_=lnT[:, ft, sl], func=AF.Copy,
                                bias=0.0, scale=pscale,
                            )
                        t = scr.tile([P, 512], BF, tag="t")
                        nc.vector.tensor_mul(out=t, in0=comp, in1=mrl)
                        msk = scr.tile([P, 512], mybir.dt.uint8, tag="msk")
                        nc.vector.tensor_scalar(
                            out=msk, in0=mrl, scalar1=0.0, scalar2=None, op0=ALU.is_gt
                        )
                        nc.vector.copy_predicated(out=dst, mask=msk, data=t)

            kT = es_kT.enter_context(tc.tile_pool(name="kT", bufs=1)).tile(
                [P, ND, 2048], BF, tag="kT"
            )
            wpk, wmk = load_w(wp_d["k"]), load_w(wm_d["k"])
            t_branch(kT, wpk, wmk, nck, bias_t.get("k"), 2048, 1.0, 1.0)
            qT = es_qT.enter_context(tc.tile_pool(name="qT", bufs=1)).tile(
                [P, ND, NJ * P], BF, tag="qT"
            )
            wpq, wmq = load_w(wp_d["q"]), load_w(wm_d["q"])
            t_branch(qT, wpq, wmq, ncq, bias_t.get("q"), NJ * P, SCALE, SCALE)
            es_lnT.close()
            es_psB.close()

        # prefetch o weights (slots free during attention)
        wpo, wmo = load_w(wp_d["o"]), load_w(wm_d["o"])

        # ================= phase E: attention =================
        attn = es_attn.enter_context(
            tc.tile_pool(name="attn", bufs=1, side="right")
        ).tile([P, NJ, D], F32, tag="attn")
        with ExitStack() as esE:
            scrE = esE.enter_context(tc.tile_pool(name="scrE", bufs=2))
            ps_strip = esE.enter_context(
                tc.tile_pool(name="psStrip", bufs=1, space="PSUM")
            )
            ps_pv = esE.enter_context(tc.tile_pool(name="psPv", bufs=1, space="PSUM"))
            ps_ptr = esE.enter_context(
                tc.tile_pool(name="psPtr", bufs=2, space="PSUM")
            )
            for j in range(NJ):
                mb = _nblocks(j)
                nb = 2 * mb  # total 256-wide key blocks (mine + partner)
                strip = ps_strip.tile([P, 2048], F32, tag="strip")
                for ib in range(nb):
                    base = ib * 256 if ib < mb else 1024 + (ib - mb) * 256
                    ssl = slice(ib * 256, (ib + 1) * 256)
                    for dc in range(ND):
                        nc.tensor.matmul(
                            strip[:, ssl],
                            qT[:, dc, j * P : (j + 1) * P],
                            kT[:, dc, base : base + 256],
                            start=(dc == 0),
                            stop=(dc == ND - 1),
                        )
                # masks on the last block of each region
                m_mine = mme if j % 2 == 0 else mmo
                m_part = mpe if j % 2 == 0 else mpo
                nc.vector.tensor_tensor(
                    out=strip[:, (mb - 1) * 256 : mb * 256],
                    in0=strip[:, (mb - 1) * 256 : mb * 256],
                    in1=m_mine, op=ALU.add,
                )
                nc.vector.tensor_tensor(
                    out=strip[:, (nb - 1) * 256 : nb * 256],
                    in0=strip[:, (nb - 1) * 256 : nb * 256],
                    in1=m_part, op=ALU.add,
                )
                nm = scrE.tile([P, 1], F32, tag="nm")
                nc.vector.reduce_max(
                    out=nm, in_=strip[:, : nb * 256], axis=AX.X, negate=True
                )
                p_sb = scrE.tile([P, 2048], BF, tag="p")
                l_parts = scrE.tile([P, 4], F32, tag="lp")
                for i in range(nb // 2):
                    nc.scalar.activation(
                        out=p_sb[:, i * 512 : (i + 1) * 512],
                        in_=strip[:, i * 512 : (i + 1) * 512],
                        func=AF.Exp, bias=nm, scale=1.0,
                        accum_out=l_parts[:, i : i + 1],
                    )
                lsum = scrE.tile([P, 1], F32, tag="l")
                nc.vector.reduce_sum(out=lsum, in_=l_parts[:, : nb // 2], axis=AX.X)
                rinv = scrE.tile([P, 1], F32, tag="r")
                nc.vector.reciprocal(out=rinv, in_=lsum)

                pv = ps_pv.tile([P, D], F32, tag="pv")
                for ib in range(nb):
                    for half in range(2):
                        kc = ib * 2 + half  # 128-chunk within strip
                        v_kc = kc if ib < mb else 8 + (ib - mb) * 2 + half
                        pT_ps = ps_ptr.tile([P, P], BF, tag="ptr")
                        nc.tensor.transpose(
                            out=pT_ps, in_=p_sb[:, kc * P : (kc + 1) * P],
                            identity=idb,
                        )
                        pT_sb = scrE.tile([P, P], BF, tag="pt")
                        nc.vector.tensor_copy(out=pT_sb, in_=pT_ps)
                        for vh in range(2):
                            nc.tensor.matmul(
                                pv[:, vh * 512 : (vh + 1) * 512],
                                pT_sb,
                                vN[:, v_kc, vh * 512 : (vh + 1) * 512],
                                start=(ib == 0 and half == 0),
                                stop=(ib == nb - 1 and half == 1),
                            )
                for vh in range(2):
                    nc.scalar.activation(
                        out=attn[:, j, vh * 512 : (vh + 1) * 512],
                        in_=pv[:, vh * 512 : (vh + 1) * 512],
                        func=AF.Copy, bias=0.0, scale=rinv,
                    )
        es_qT.close()
        es_kT.close()
        es_v.close()
        es_scrB.close()

        # ================= phase F: o branch + residual =================
        with ExitStack() as esF:
            scrF = esF.enter_context(tc.tile_pool(name="scrF", bufs=3))
            psF = esF.enter_context(tc.tile_pool(name="psF", bufs=4, space="PSUM"))
            psFt = esF.enter_context(tc.tile_pool(name="psFt", bufs=2, space="PSUM"))
            for rt in range(NJ):
                attnT = scrF.tile([P, ND, P], BF, tag="at")
                for half in range(2):
                    trp = psFt.tile([P, 512], F32, tag="tr")
                    for t in range(4):
                        dc = half * 4 + t
                        nc.tensor.transpose(
                            out=trp[:, t * P : (t + 1) * P],
                            in_=attn[:, rt, dc * P : (dc + 1) * P],
                            identity=idf,
                        )
                    nc.vector.tensor_copy(
                        out=attnT[:, half * 4 : (half + 1) * 4, :],
                        in_=trp[:].rearrange("p (a b) -> p a b", b=P),
                    )
                xres = scrF.tile([P, D], F32, tag="xr")
                nc.sync.dma_start(out=xres, in_=x_d[rt * P : (rt + 1) * P, :])
                outsb = scrF.tile([P, D], F32, tag="ou")
                for nh in range(2):
                    sl = slice(nh * 512, (nh + 1) * 512)
                    rl = psF.tile([P, 512], F32, tag="mm")
                    cp = psF.tile([P, 512], F32, tag="mm")
                    for dc in range(ND):
                        nc.tensor.matmul(rl, attnT[:, dc, :], wpo[:, dc, sl],
                                         start=(dc == 0), stop=(dc == ND - 1))
                    for dc in range(ND):
                        nc.tensor.matmul(cp, attnT[:, dc, :], wmo[:, dc, sl],
                                         start=(dc == 0), stop=(dc == ND - 1))
                    if "o" in bias_t:
                        nc.vector.tensor_tensor(
                            out=cp, in0=cp, in1=bias_t["o"][:, sl], op=ALU.add
                        )
                    nc.vector.tensor_tensor(
                        out=rl, in0=rl, in1=cob[:, sl], op=ALU.subtract
                    )
                    mrl = scrF.tile([P, 512], F32, tag="mrl")
                    nc.scalar.activation(out=mrl, in_=rl, func=AF.Relu, bias=0.0)
                    comp = scrF.tile([P, 512], F32, tag="comp")
                    nc.scalar.activation(out=comp, in_=cp, func=AF.Silu, bias=0.0)
                    omix = scrF.tile([P, 512], F32, tag="om")
                    nc.scalar.copy(out=omix, in_=attn[:, rt, sl])
                    t = scrF.tile([P, 512], F32, tag="t")
                    nc.vector.tensor_mul(out=t, in0=comp, in1=mrl)
                    msk = scrF.tile([P, 512], mybir.dt.uint8, tag="msk")
                    nc.vector.tensor_scalar(
                        out=msk, in0=mrl, scalar1=0.0, scalar2=None, op0=ALU.is_gt
                    )
                    nc.vector.copy_predicated(out=omix, mask=msk, data=t)
                    nc.vector.tensor_tensor(
                        out=outsb[:, sl], in0=omix, in1=xres[:, sl], op=ALU.add
                    )
                nc.sync.dma_start(out=out_d[rt * P : (rt + 1) * P, :], in_=outsb)
        es_attn.close()

    nc.compile()
    return nc


_NC_CACHE = {}


def _get_nc(flags):
    if flags not in _NC_CACHE:
        _NC_CACHE[flags] = _build(*flags)
    return _NC_CACHE[flags]


def _host_masks():
    i = np.arange(P, dtype=np.int64)[:, None]
    c = np.arange(256, dtype=np.int64)[None, :]
    neg = np.float32(NEG)
    zero = np.float32(0.0)
    m_even = np.where(c <= i, zero, neg).astype(np.float32)
    m_odd = np.where((c < P) | ((c - P) <= i), zero, neg).astype(np.float32)
    half_mask = np.ascontiguousarray(
        np.broadcast_to(np.where(c < P, zero, neg), (P, 256))
    ).astype(np.float32)  # second half masked
    full_mask = np.full((P, 256), neg, dtype=np.float32)
    zeros = np.zeros((P, 256), dtype=np.float32)
    # partner-region last-block masks, by (j parity, h)
    p_even = {0: full_mask, 1: half_mask}
    p_odd = {0: half_mask, 1: zeros}
    return m_even, m_odd, p_even, p_odd


def _prep_shared(q_mu_w, q_mu_b, q_proto, q_gate, k_mu_w, k_mu_b, k_proto, k_gate,
                 v_mu_w, v_mu_b, v_proto, v_gate, o_mu_w, o_mu_b, o_proto, o_gate,
                 ln_g, ln_b, flags):
    f32 = np.float32
    sh = {}
    for br, proto, mu_w in (("q", q_proto, q_mu_w), ("k", k_proto, k_mu_w),
                            ("v", v_proto, v_mu_w), ("o", o_proto, o_mu_w)):
        sh[f"wp_{br}"] = np.ascontiguousarray(
            (np.asarray(proto, f32).T * f32(SCALE))).astype(BF16)
        sh[f"wm_{br}"] = np.ascontiguousarray(np.asarray(mu_w, f32).T).astype(BF16)

    def cost(gate):
        g = np.asarray(gate, f32)
        return (g / (np.max(np.abs(g)) + f32(1e-9))).astype(f32)

    sh["ncost_q"] = (-cost(q_gate) * f32(SCALE)).astype(f32)
    sh["ncost_k"] = (-cost(k_gate)).astype(f32)
    sh["cost_v"] = cost(v_gate)
    sh["cost_o"] = cost(o_gate)
    m_even, m_odd, p_even, p_odd = _host_masks()
    sh["mask_m_even"] = m_even
    sh["mask_m_odd"] = m_odd
    gb_trivial, bq, bk, bv, bo = flags
    if not gb_trivial:
        sh["ln_g"] = np.asarray(ln_g, f32)
        sh["ln_b"] = np.asarray(ln_b, f32)
    for br, flag, b in (("q", bq, q_mu_b), ("k", bk, k_mu_b),
                        ("v", bv, v_mu_b), ("o", bo, o_mu_b)):
        if flag:
            sh[f"bias_{br}"] = np.asarray(b, f32)
    return sh, p_even, p_odd


def _run(inputs_kw, trace=False, **kw):
    f32 = np.float32
    x = np.asarray(inputs_kw["x"], f32)
    ln_g = np.asarray(inputs_kw["ln_g"], f32)
    ln_b = np.asarray(inputs_kw["ln_b"], f32)
    flags = (
        bool(np.all(ln_g == 1.0) and np.all(ln_b == 0.0)),
        bool(np.any(inputs_kw["q_mu_b"])),
        bool(np.any(inputs_kw["k_mu_b"])),
        bool(np.any(inputs_kw["v_mu_b"])),
        bool(np.any(inputs_kw["o_mu_b"])),
    )
    nc = _get_nc(flags)
    sh, p_even, p_odd = _prep_shared(
        inputs_kw["q_mu_w"], inputs_kw["q_mu_b"], inputs_kw["q_proto"],
        inputs_kw["q_gate"], inputs_kw["k_mu_w"], inputs_kw["k_mu_b"],
        inputs_kw["k_proto"], inputs_kw["k_gate"], inputs_kw["v_mu_w"],
        inputs_kw["v_mu_b"], inputs_kw["v_proto"], inputs_kw["v_gate"],
        inputs_kw["o_mu_w"], inputs_kw["o_mu_b"], inputs_kw["o_proto"],
        inputs_kw["o_gate"], ln_g, ln_b, flags,
    )
    in_maps = []
    for c in range(8):
        b, h = divmod(c, 2)
        order = [2 * j + h for j in range(NJ)] + [2 * j + 1 - h for j in range(NJ)]
        xb = np.asarray(x[b], f32).reshape(NT, P, D)
        x_perm = np.ascontiguousarray(xb[order].reshape(2048, D))
        m = dict(sh)
        m["x"] = x_perm
        m["mask_p_even"] = p_even[h]
        m["mask_p_odd"] = p_odd[h]
        in_maps.append(m)
    bk_res = run_bass_kernel_spmd(nc, in_maps, list(range(8)), trace=trace, **kw)
    out = np.empty((4, 2048, D), f32)
    for c in range(8):
        b, h = divmod(c, 2)
        oc = np.asarray(bk_res.results[c]["out"], f32).reshape(NJ, P, D)
        out[b].reshape(NT, P, D)[[2 * j + h for j in range(NJ)]] = oc
    return out, bk_res


def kernel(**inputs):
    out, _ = _run(inputs, trace=False)
    return out


def kernel_traced(**inputs):
    return _run(inputs, trace=True)
